# revision 1
# baseline (speedup 1.0000x reference)
"""CommandModel LSTM-decode kernel: builder + host prep + numpy reference.

Design (8 cores, one TRN2 chip):
- vocab sharded 8-way (1024/core) for logits + local argmax
- hidden sharded 4-way (128/core-pair, pairs duplicate) for LSTM state
- LSTM state kept transposed ([hidden, batch]); gates computed as
  gates.T [128 gate-units, 64 batch] with W stationary (fp32-friendly)
- cross-core exchange via AllGather collectives over DRAM bounce buffers
  (h-shards: 4-rank even/odd groups; candidates: 8-rank)
- sampling: gumbel noise precomputed on host (bit-exact jax threefry),
  streamed from HBM; argmax via DVE max/max_index; global argmax via
  PE-transposed candidate table
"""
import numpy as np
import concourse.bass as bass
import concourse.tile as tile
from concourse import bacc, mybir
from concourse.tile import add_dep_helper

F32 = mybir.dt.float32
U32 = mybir.dt.uint32

B = 64          # batch
H = 512         # hidden
E = 256         # embed
V = 8192        # vocab
NCORES = 8
HSH = H // 4    # hidden shard (per core-pair) = 128
VSH = V // NCORES  # vocab shard = 1024


def build(T: int):
    nc = bacc.Bacc("TRN2", target_bir_lowering=False, debug=False, num_devices=NCORES)

    # ---- inputs (per-core contents prepared by host) ----
    wih_d = nc.dram_tensor("wih", [128, 2, 4, 128], F32, kind="ExternalInput").ap()
    whh_d = nc.dram_tensor("whh", [128, 4, 4, 128], F32, kind="ExternalInput").ap()
    wout_d = nc.dram_tensor("wout", [128, 4, VSH], F32, kind="ExternalInput").ap()
    biasT_d = nc.dram_tensor("biasT", [128, 4], F32, kind="ExternalInput").ap()
    bias0T_d = nc.dram_tensor("bias0T", [128, 4], F32, kind="ExternalInput").ap()
    obsT_d = nc.dram_tensor("obsT", [128, 4, B], F32, kind="ExternalInput").ap()
    c0T_d = nc.dram_tensor("c0T", [128, B], F32, kind="ExternalInput").ap()
    emb_d = nc.dram_tensor("emb", [V, E], F32, kind="ExternalInput").ap()
    gumb_d = nc.dram_tensor("gumb", [200, B, VSH], F32, kind="ExternalInput").ap()
    voff_d = nc.dram_tensor("voff", [B, 1], F32, kind="ExternalInput").ap()
    ident_d = nc.dram_tensor("ident", [128, 128], F32, kind="ExternalInput").ap()

    # ---- outputs ----
    logits_d = nc.dram_tensor("logits", [200, B, VSH], F32, kind="ExternalOutput").ap()
    tokens_d = nc.dram_tensor("tokens", [B, T], U32, kind="ExternalOutput").ap()

    with tile.TileContext(nc) as tc:
        with tc.tile_pool(name="wp", bufs=1) as wp, \
             tc.tile_pool(name="xp", bufs=2) as xp, \
             tc.tile_pool(name="sp", bufs=2) as sp, \
             tc.tile_pool(name="gp", bufs=3) as gp, \
             tc.tile_pool(name="lp", bufs=2) as lp, \
             tc.tile_pool(name="cp", bufs=2) as cp, \
             tc.tile_pool(name="ppg", bufs=1, space="PSUM") as ppg, \
             tc.tile_pool(name="ppx", bufs=2, space="PSUM") as ppx, \
             tc.tile_pool(name="ppl", bufs=1, space="PSUM") as ppl:

            # ---- resident weights/state ----
            wih_s = wp.tile([128, 2, 4, 128], F32, tag="wih")
            whh_s = wp.tile([128, 4, 4, 128], F32, tag="whh")
            wout_s = wp.tile([128, 4, VSH], F32, tag="wout")
            biasT_s = wp.tile([128, 4], F32, tag="biasT")
            bias0T_s = wp.tile([128, 4], F32, tag="bias0T")
            ident_s = wp.tile([128, 128], F32, tag="ident")
            voff_s = wp.tile([B, 1], F32, tag="voff")
            hT = wp.tile([128, 4, B], F32, tag="hT")        # full h.T, rewritten per step
            cT = wp.tile([128, B], F32, tag="cT")           # pair shard of c.T
            toks = wp.tile([B, T], U32, tag="toks")
            big = wp.tile([B, 8], F32, tag="big")

            nc.sync.dma_start(wih_s[:], wih_d)
            nc.sync.dma_start(whh_s[:], whh_d)
            nc.sync.dma_start(wout_s[:], wout_d)
            nc.sync.dma_start(biasT_s[:], biasT_d)
            nc.sync.dma_start(bias0T_s[:], bias0T_d)
            nc.sync.dma_start(ident_s[:], ident_d)
            nc.sync.dma_start(voff_s[:], voff_d)
            nc.sync.dma_start(hT[:], obsT_d)
            nc.sync.dma_start(cT[:], c0T_d)
            nc.vector.memset(big[:], 1.0e9)

            tok_u32 = None  # [B, 1] u32 from previous step

            for t in range(T):
                # ---------- x projection (skipped at t=0: folded into bias0) ----------
                xT = None
                if t > 0:
                    x_b = xp.tile([B, E], F32, tag="x_b")
                    nc.gpsimd.indirect_dma_start(
                        out=x_b[:],
                        out_offset=None,
                        in_=emb_d,
                        in_offset=bass.IndirectOffsetOnAxis(ap=tok_u32[:, 0:1], axis=0),
                    )
                    xT = xp.tile([128, 2, B], F32, tag="xT")
                    for j in range(2):
                        pt = ppx.tile([128, B], F32, tag="px")
                        nc.tensor.transpose(pt[:], x_b[:, 128 * j:128 * (j + 1)],
                                            ident_s[:B, :B])
                        nc.scalar.copy(xT[:, j, :], pt[:])

                # ---------- gates + LSTM state (transposed layout) ----------
                bias_src = bias0T_s if t == 0 else biasT_s
                sig_i = sp.tile([128, B], F32, tag="sig_i")
                sig_f = sp.tile([128, B], F32, tag="sig_f")
                tanh_g = sp.tile([128, B], F32, tag="tanh_g")
                sig_o = sp.tile([128, B], F32, tag="sig_o")
                act_out = [sig_i, sig_f, tanh_g, sig_o]
                act_fn = [mybir.ActivationFunctionType.Sigmoid,
                          mybir.ActivationFunctionType.Sigmoid,
                          mybir.ActivationFunctionType.Tanh,
                          mybir.ActivationFunctionType.Sigmoid]
                for g in range(4):
                    pg = ppg.tile([128, B], F32, tag=f"pg{g}")
                    first = True
                    if t > 0:
                        for kc in range(2):
                            nc.tensor.matmul(pg[:], wih_s[:, kc, g, :], xT[:, kc, :],
                                             start=first, stop=False)
                            first = False
                    for kc in range(4):
                        nc.tensor.matmul(pg[:], whh_s[:, kc, g, :], hT[:, kc, :],
                                         start=first, stop=(kc == 3))
                        first = False
                    nc.scalar.activation(act_out[g][:], pg[:], act_fn[g],
                                         bias=bias_src[:, g:g + 1])

                m1 = sp.tile([128, B], F32, tag="m1")
                m2 = sp.tile([128, B], F32, tag="m2")
                nc.vector.tensor_mul(m1[:], sig_f[:], cT[:])
                nc.vector.tensor_mul(m2[:], sig_i[:], tanh_g[:])
                nc.vector.tensor_add(cT[:], m1[:], m2[:])
                th_c = sp.tile([128, B], F32, tag="th_c")
                nc.scalar.activation(th_c[:], cT[:], mybir.ActivationFunctionType.Tanh)
                hsh = sp.tile([128, B], F32, tag="hsh")
                nc.vector.tensor_mul(hsh[:], sig_o[:], th_c[:])

                # ---------- h AllGather (4-rank even/odd groups) ----------
                cinh = nc.dram_tensor(f"cinh{t}", [128, B], F32, kind="Internal").ap()
                couth = nc.dram_tensor(f"couth{t}", [4 * 128, B], F32,
                                       kind="Internal").ap()
                d1 = nc.sync.dma_start(cinh, hsh[:])
                cch = nc.gpsimd.collective_compute(
                    "AllGather", mybir.AluOpType.bypass,
                    replica_groups=[[0, 2, 4, 6], [1, 3, 5, 7]],
                    ins=[cinh], outs=[couth])
                add_dep_helper(cch.ins, d1.ins, reason="cc after send dma")
                for j in range(4):
                    dj = nc.sync.dma_start(hT[:, j, :], couth[128 * j:128 * (j + 1), :])
                    add_dep_helper(dj.ins, cch.ins, reason="recv after cc")

                # ---------- logits (batch-M layout, fp32) ----------
                pl = ppl.tile([B, VSH], F32, tag="pl")
                for kc in range(4):
                    for nn_ in range(2):
                        nc.tensor.matmul(pl[:, 512 * nn_:512 * (nn_ + 1)],
                                         hT[:, kc, 0:B],
                                         wout_s[:, kc, 512 * nn_:512 * (nn_ + 1)],
                                         start=(kc == 0), stop=(kc == 3))

                lsb = lp.tile([B, VSH], F32, tag="lsb")
                nc.scalar.copy(lsb[:], pl[:])
                nc.sync.dma_start(logits_d[t], lsb[:])

                # ---------- sampling: local argmax ----------
                gmt = gp.tile([B, VSH], F32, tag="gmt")
                nc.sync.dma_start(gmt[:], gumb_d[t])
                gadd = gp.tile([B, VSH], F32, tag="gadd")
                nc.vector.tensor_add(gadd[:], pl[:], gmt[:])
                mx8 = cp.tile([B, 8], F32, tag="mx8")
                mi8 = cp.tile([B, 8], U32, tag="mi8")
                nc.vector.max(out=mx8[:], in_=gadd[:])
                nc.vector.max_index(out=mi8[:], in_max=mx8[:], in_values=gadd[:])

                idxf = cp.tile([B, 1], F32, tag="idxf")
                nc.vector.tensor_copy(idxf[:], mi8[:, 0:1])      # u32 -> f32
                cand = cp.tile([B, 2], F32, tag="cand")
                nc.vector.tensor_add(cand[:, 1:2], idxf[:], voff_s[:])
                nc.vector.tensor_copy(cand[:, 0:1], mx8[:, 0:1])

                # ---------- candidate AllGather (8-rank) ----------
                pc2 = ppx.tile([128, B], F32, tag="px")
                nc.tensor.transpose(pc2[0:2, 0:B], cand[:], ident_s[:B, :B])
                c2 = cp.tile([2, B], F32, tag="c2")
                nc.scalar.copy(c2[:], pc2[0:2, 0:B])
                cinc = nc.dram_tensor(f"cinc{t}", [2, B], F32, kind="Internal").ap()
                coutc = nc.dram_tensor(f"coutc{t}", [16, B], F32, kind="Internal",
                                       addr_space="Shared").ap()
                d2 = nc.sync.dma_start(cinc, c2[:])
                ccc = nc.gpsimd.collective_compute(
                    "AllGather", mybir.AluOpType.bypass,
                    replica_groups=[list(range(NCORES))],
                    ins=[cinc], outs=[coutc])
                add_dep_helper(ccc.ins, d2.ins, reason="cc after send dma")
                r16 = cp.tile([16, B], F32, tag="r16")
                d3 = nc.sync.dma_start(r16[:], coutc)
                add_dep_helper(d3.ins, ccc.ins, reason="recv after cc")

                # ---------- global argmax ----------
                pr = ppx.tile([128, B], F32, tag="px")
                nc.tensor.transpose(pr[0:B, 0:16], r16[:], ident_s[:16, :16])
                t16 = cp.tile([B, 16], F32, tag="t16")
                nc.vector.tensor_copy(t16[:], pr[0:B, 0:16])
                tv = t16[:].rearrange("p (s two) -> p s two", two=2)
                vals = tv[:, :, 0]
                idxs = tv[:, :, 1]
                gmax = cp.tile([B, 1], F32, tag="gmax")
                nc.vector.tensor_reduce(gmax[:], vals, axis=mybir.AxisListType.X,
                                        op=mybir.AluOpType.max)
                eq = cp.tile([B, 8], mybir.dt.uint8, tag="eq")
                nc.vector.tensor_tensor(eq[:], vals, gmax[:].to_broadcast([B, 8]),
                                        op=mybir.AluOpType.is_equal)
                selv = cp.tile([B, 8], F32, tag="selv")
                nc.vector.select(selv[:], eq[:], idxs, big[:])
                tokf = cp.tile([B, 1], F32, tag="tokf")
                nc.vector.tensor_reduce(tokf[:], selv[:], axis=mybir.AxisListType.X,
                                        op=mybir.AluOpType.min)
                tok_u32 = cp.tile([B, 1], U32, tag="tok_u32")
                nc.vector.tensor_copy(tok_u32[:], tokf[:])  # f32 -> u32
                nc.vector.tensor_copy(toks[:, t:t + 1], tok_u32[:])

            nc.sync.dma_start(tokens_d, toks[:])

    nc.compile()
    return nc


# ---------------- host-side preparation ----------------

def prep_inputs(inputs: dict, T: int) -> list[dict]:
    obs = np.asarray(inputs["obs_batch"], np.float32)
    emb = np.asarray(inputs["emb_table"], np.float32)
    W_ih = np.asarray(inputs["W_ih"], np.float32)
    W_hh = np.asarray(inputs["W_hh"], np.float32)
    b_ih = np.asarray(inputs["b_ih"], np.float32)
    b_hh = np.asarray(inputs["b_hh"], np.float32)
    W_out = np.asarray(inputs["W_out"], np.float32)
    b_out = np.asarray(inputs["b_out"], np.float32)

    gum = gumbel_noise(200)  # fixed shape so transfers don't vary with T

    bias = b_ih + b_hh
    x0 = emb[0]  # START_TOKEN = 0
    gx0 = W_ih @ x0 + bias  # [2048]

    ident = np.eye(128, dtype=np.float32)

    in_maps = []
    for c in range(NCORES):
        p = c // 2
        # gate rows for hidden shard p, as [g, m] -> row g*H + p*128 + m
        rows = np.stack([np.arange(g * H + p * HSH, g * H + p * HSH + HSH)
                         for g in range(4)])  # [4, 128]
        wih_c = np.transpose(
            W_ih[rows][:, :, :].reshape(4, HSH, 2, 128), (3, 2, 0, 1)
        ).copy()  # [kp=128, kc=2, g=4, m=128]
        whh_c = np.transpose(
            W_hh[rows][:, :, :].reshape(4, HSH, 4, 128), (3, 2, 0, 1)
        ).copy()  # [128, 4, 4, 128]
        wout_c = np.transpose(
            W_out[VSH * c:VSH * (c + 1)].reshape(VSH, 4, 128), (2, 1, 0)
        ).copy()  # [kp=128, kc=4, n=1024]
        biasT = bias[rows].T.copy()   # [m=128, g=4]
        bias0T = gx0[rows].T.copy()
        obsT = np.transpose(obs.reshape(B, 4, 128), (2, 1, 0)).copy()  # [128, 4, B]
        c0T = obsT[:, p, :].copy()
        gumb_c = (gum[:, :, VSH * c:VSH * (c + 1)]
                  + b_out[None, None, VSH * c:VSH * (c + 1)]).copy()
        voff = np.full((B, 1), VSH * c, np.float32)
        in_maps.append({
            "wih": wih_c, "whh": whh_c, "wout": wout_c,
            "biasT": biasT, "bias0T": bias0T,
            "obsT": obsT, "c0T": c0T, "emb": emb,
            "gumb": gumb_c, "voff": voff, "ident": ident,
        })
    return in_maps


_GUMBEL_SRC = """
import sys, numpy as np
import jax
jax.config.update("jax_platforms", "cpu")
import jax.numpy as jnp
T = int(sys.argv[1]); out = sys.argv[2]
B, V = 64, 8192
keys = jax.random.split(jax.random.key(42), 200)[:T]
one = jax.jit(lambda k: jax.random.gumbel(k, (B, V), jnp.float32))
gum = np.stack([np.asarray(one(keys[i])) for i in range(T)])
np.save(out, gum)
"""


def gumbel_noise(T: int) -> np.ndarray:
    """Gumbel noise bit-exact with the CPU-jax reference (subprocess pins CPU)."""
    import os, subprocess, sys, tempfile
    with tempfile.TemporaryDirectory() as td:
        out = os.path.join(td, "gum.npy")
        env = dict(os.environ, JAX_PLATFORMS="cpu")
        subprocess.run([sys.executable, "-c", _GUMBEL_SRC, str(T), out],
                       env=env, check=True, capture_output=True)
        return np.load(out)


def assemble_outputs(results: list[dict], inputs: dict, T: int):
    b_out = np.asarray(inputs["b_out"], np.float32)
    logits = np.concatenate([r["logits"] for r in results], axis=2)  # [T, B, V]
    logits = logits + b_out[None, None, :]
    tokens = results[0]["tokens"].T.astype(np.int32)  # [T, B]
    return tokens, logits




# ---------------- harness entry point ----------------

T_STEPS = 200
_CACHE = {}


def kernel(**inputs) -> tuple:
    """Full-input entry point: shards across the 8 NeuronCores internally,
    runs the SPMD Bass kernel, and reassembles full-shape outputs.

    Returns (tokens [200, 64] int32, logits [200, 64, 8192] float32),
    matching reference.reference().
    """
    from concourse.bass_utils import run_bass_kernel_spmd

    inputs = {k: np.asarray(v) for k, v in inputs.items()}
    if "nc" not in _CACHE:
        _CACHE["nc"] = build(T_STEPS)
    nc = _CACHE["nc"]
    in_maps = prep_inputs(inputs, T_STEPS)
    res = run_bass_kernel_spmd(nc, in_maps, core_ids=list(range(NCORES)))
    results = [{"logits": res.results[c]["logits"],
                "tokens": res.results[c]["tokens"]} for c in range(NCORES)]
    return assemble_outputs(results, inputs, T_STEPS)
